# revision 34
# baseline (speedup 1.0000x reference)
"""GQA multi-head attention (B=2, S=2048, D=2048, 32 q-heads / 8 kv-heads)
on 8 Trainium2 NeuronCores.

Sharding: tensor-parallel over kv-head groups. Core c owns kv head c and its
4 query heads: Wq column-shard [2048, 256], Wk/Wv column-shard [2048, 64],
Wo row-shard [256, 2048]. Each core computes a full-shape partial output
(its heads' contribution through Wo); the host sums the 8 partials.

Numerics: dense projections (q/k/v and the out-projection) run as fp8e4
DoubleRow matmuls with hi+lo error compensation: x ~ hi + lo in fp8,
out = xh*Wh + xh*Wl + xl*Wh (0.75x the bf16 PE cost; dropped lo*lo term is
~2^-8 relative). Host pre-scales weights x64 so fp8 lo parts don't denormal;
ctx is scaled x64 on-device (the vsb ones column is 1/64 so the softmax
reciprocal supplies the 64); 1/4096 is folded into the out-proj psum copies.
The ctx hi/lo split for the fp8 out-proj runs on the idle Pool engine
(bit-exact vs host split; verified on the NEFF path). The last (b=1,qc=3)
out-proj stays bf16 so the end tail has no Pool dependency.

Schedule (the kernel is ACT-exp bound at ~266us; PE busy ~275us):
- Lead-in is DMA-ordered: wk, wq, q(0,0), K(0,0..3) first; scores slot 0
  chases the K-chunk DMAs at kt granularity. V(b0) + q(0,1) + Wo stream
  behind them; V-projections are pumped as filler in slots 1-4 (first ctx
  is gated to slot 4 when V(b0) has landed).
- Per-slot round-robin of small PE pieces (0.3-1.3us) between the 8
  scores/exp steps keeps the ACT queue fed; ctx lags scores by 2 slots
  (exp quadruple-buffered to absorb the V-gated start), out-proj pieces
  run eagerly as soon as each (b,qc)'s ctx is split to fp8.
- Tail: the last slot chases ctx for qt0/qt2 in two psum banks at kt
  granularity behind the exp stream; qt1/qt3 + the 4 bf16 out-proj tiles
  are all that remains after the final exp.

DMA XBAR transpose races on the compiled NEFF path - PE transposes only.
"""
from collections import deque
from contextlib import ExitStack

import numpy as np
import ml_dtypes

import jax

try:
    jax.config.update("jax_compilation_cache_dir", "/tmp/jax_bass_cache")
    jax.config.update("jax_persistent_cache_min_compile_time_secs", 1.0)
except Exception:
    pass

from jax.sharding import Mesh, PartitionSpec, NamedSharding
from jax.experimental.shard_map import shard_map

import concourse.bass as bass
import concourse.mybir as mybir
import concourse.tile as tile
from concourse import bacc, bass2jax
from concourse.masks import make_identity

BF16 = mybir.dt.bfloat16
FP8 = mybir.dt.float8e4
F32 = mybir.dt.float32
AF = mybir.ActivationFunctionType
DR = mybir.MatmulPerfMode.DoubleRow
SUB = mybir.AluOpType.subtract

B, S, DM = 2, 2048, 2048
HKV, G, DH = 8, 4, 64
DQ = G * DH            # 256: per-core q-projection width
NC = 8
DT = DM // 128         # 16 contraction tiles
BS = B * S             # 4096
SCALE = 1.0 / 8.0      # 1/sqrt(64)

_cache = {}


def _emit(ctx, tc, qTh, qTl, kTh, kTl, vTh, vTl, wqh, wql, wkh, wkl, wvh,
          wvl, woh, wol, wo, out):
    nc = tc.nc

    pp = ctx.enter_context(tc.tile_pool(name="persist", bufs=1))
    wqh_sb = pp.tile([128, DT, DQ], FP8, tag="wqh")
    wql_sb = pp.tile([128, DT, DQ], FP8, tag="wql")
    wkh_sb = pp.tile([128, DT, DH], FP8, tag="wkh")
    wkl_sb = pp.tile([128, DT, DH], FP8, tag="wkl")
    wvh_sb = pp.tile([128, DT, DH], FP8, tag="wvh")
    wvl_sb = pp.tile([128, DT, DH], FP8, tag="wvl")
    woh_sb = pp.tile([128, 2, DM], FP8, tag="woh")
    wol_sb = pp.tile([128, 2, DM], FP8, tag="wol")
    wo_sb = pp.tile([128, 2, DM], BF16, tag="wo")
    qtp = pp.tile([128, 2, BS], BF16, tag="qtp")    # QT pairs [p, m, b*S+s]
    ktd = pp.tile([128, BS], BF16, tag="ktd")       # KT duplicated both halves
    vsb = pp.tile([128, BS // 128, DH + 1], BF16, tag="vsb")  # V + 1/64 col
    ident = pp.tile([128, 128], BF16, tag="ident")
    make_identity(nc, ident[:])
    nc.gpsimd.memset(vsb[:, :, DH], 1.0 / 64.0)

    kst = ctx.enter_context(tc.tile_pool(name="kst", bufs=4))
    vst = ctx.enter_context(tc.tile_pool(name="vst", bufs=4))
    qst = ctx.enter_context(tc.tile_pool(name="qst", bufs=3))
    expp = ctx.enter_context(tc.tile_pool(name="expp", bufs=4))
    ctxs = ctx.enter_context(tc.tile_pool(name="ctxs", bufs=2))
    c8p = ctx.enter_context(tc.tile_pool(name="c8p", bufs=3))
    c8u = ctx.enter_context(tc.tile_pool(name="c8u", bufs=1))
    smal = ctx.enter_context(tc.tile_pool(name="small", bufs=2))
    outp = ctx.enter_context(tc.tile_pool(name="outp", bufs=3))
    psum = ctx.enter_context(tc.tile_pool(name="psum", bufs=1, space="PSUM"))

    # ---- weight DMAs (SP queue; order = priority) --------------------------
    nc.sync.dma_start(wkh_sb[:], wkh)
    nc.sync.dma_start(wkl_sb[:], wkl)
    nc.sync.dma_start(wqh_sb[:], wqh)
    nc.sync.dma_start(wql_sb[:], wql)

    # ---- work-piece pump ---------------------------------------------------
    # Generators yield their nominal PE cost (ns); the pump meters emission
    # so the exp stream is never starved and no backlog dumps into the tail.
    work = deque()

    def pump_one():
        while work:
            try:
                next(work[0])
                return True
            except StopIteration:
                work.popleft()
        return False

    def pump_budget(budget):
        spent = 0
        while work and spent < budget:
            try:
                c = next(work[0])
                spent += c if c else 400
            except StopIteration:
                work.popleft()
        return spent

    def run_gen(g):
        for _ in g:
            pass

    def advance(g):
        return lambda: next(g, None)

    # ---- DMA emitters ------------------------------------------------------
    # K/V stage in half-chunks (dt 0-7 / 8-15) so the a-half frees mid-gen
    # and the next chunk's DMA streams behind the projection.
    def k_dma(b, qc):
        so = qc * 512
        chs = []
        for half in range(2):
            for nm, src in (("kh", kTh), ("kl", kTl)):
                t = kst.tile([128, DT // 2, 512], FP8, tag="kst",
                             name=f"{nm}{half}_{b}_{qc}")
                nc.sync.dma_start(
                    t[:],
                    src[b].rearrange("(dt p) s -> p dt s", p=128)
                    [:, 8 * half:8 * half + 8, so:so + 512])
                chs.append(t)
        return chs

    def v_dma(b, qc):
        so = qc * 512
        chs = []
        for half in range(2):
            for nm, src in (("vh", vTh), ("vl", vTl)):
                t = vst.tile([128, DT // 2, 512], FP8, tag="vst",
                             name=f"{nm}{half}_{b}_{qc}")
                nc.sync.dma_start(
                    t[:],
                    src[b].rearrange("(dt p) s -> p dt s", p=128)
                    [:, 8 * half:8 * half + 8, so:so + 512])
                chs.append(t)
        return chs

    def q_dma(b, qc):
        so = qc * 512
        chs = []
        for nm, src in (("qh", qTh), ("ql", qTl)):
            t = qst.tile([128, DT, 512], FP8, tag="qst", name=f"{nm}_{b}_{qc}")
            nc.sync.dma_start(
                t[:],
                src[b].rearrange("(dt p) s -> p dt s", p=128)[:, :, so:so + 512])
            chs.append(t)
        return chs

    # ---- projection generators --------------------------------------------
    def _kv_matmuls(dst, chs, wh_sb, wl_sb):
        """ki-major 3-term DR accumulation over both stage halves: one psum
        accumulation group open at a time."""
        h_a, l_a, h_b, l_b = chs
        for ki in range(4):
            n = 0
            for half, (hh, ll) in enumerate(((h_a, l_a), (h_b, l_b))):
                for chv, w_sb in ((hh, wh_sb), (hh, wl_sb), (ll, wh_sb)):
                    for t in range(4):
                        wt = 8 * half + 2 * t
                        nc.tensor.matmul(
                            dst[:, ki, :],
                            chv[:, 2 * t:2 * t + 2, ki * 128:(ki + 1) * 128],
                            w_sb[:, wt:wt + 2, :],
                            start=(n == 0), stop=(n == 23),
                            perf_mode=DR)
                        n += 1
            yield 340

    def k_proj(b, qc, chs):
        """K chunk -> ktd (transposed, duplicated to both halves)."""
        bo, so = b * S, qc * 512
        kp = psum.tile([128, 4, DH], F32, tag="pa", bufs=2, name=f"kp_{b}_{qc}")
        yield from _kv_matmuls(kp, chs, wkh_sb, wkl_sb)
        ktmp = smal.tile([128, 4, DH], BF16, tag="ktmp", bufs=2,
                         name=f"ktmp_{b}_{qc}")
        nc.vector.tensor_scalar_mul(ktmp[:], kp[:], 1.0 / 64.0)
        yield 60
        for ki in range(4):
            koff = bo + so + ki * 128
            ktp = psum.tile([128, 128], BF16, tag="pc", bufs=2,
                            name=f"ktp_{b}_{qc}_{ki}")
            for half in range(2):
                nc.tensor.transpose(
                    ktp[DH * half:DH * half + DH, :], ktmp[:, ki, :],
                    ident[:], tile_position=(0, DH * half))
            nc.vector.tensor_copy(ktd[:, koff:koff + 128], ktp[:])
            if ki % 2 == 1:
                yield 280

    def v_proj(b, qc, chs):
        """V chunk -> vsb rows (keys-major, 1/64 ones col preset)."""
        vp = psum.tile([128, 4, DH], F32, tag="pa", bufs=2, name=f"vp_{b}_{qc}")
        yield from _kv_matmuls(vp, chs, wvh_sb, wvl_sb)
        nc.vector.tensor_scalar_mul(
            vsb[:, b * 16 + qc * 4:b * 16 + qc * 4 + 4, 0:DH], vp[:],
            1.0 / 64.0)
        yield 60

    def q_proj(b, qc, chs):
        qh_ch, ql_ch = chs
        bo, so = b * S, qc * 512
        terms = [(wqh_sb, qh_ch), (wqh_sb, ql_ch), (wql_sb, qh_ch)]
        for m in range(2):
            pq = psum.tile([128, 512], F32, tag="pa", bufs=2,
                           name=f"pq_{b}_{qc}_{m}")
            n = 0
            for w_sb, qch in terms:
                for t in range(DT // 2):
                    nc.tensor.matmul(
                        pq[:], w_sb[:, 2 * t:2 * t + 2, m * 128:(m + 1) * 128],
                        qch[:, 2 * t:2 * t + 2, :],
                        start=(n == 0), stop=(n == 3 * DT // 2 - 1),
                        perf_mode=DR)
                    n += 1
                    if n == 12:
                        yield 640
            nc.vector.tensor_scalar_mul(
                qtp[:, m, bo + so:bo + so + 512], pq[:], 1.0 / 64.0)
            yield 660

    # ---- ctx / out-proj ----------------------------------------------------
    cn_map = {}
    c8_map = {}     # (b, qc) -> (hi fp8, lo fp8) [128, 2, 512]
    cxs_map = {}    # (b, qc) -> bf16 ctxT staging [128, 2, 512]

    def c_st_gen(b, qc, qt):
        """fp8 3-term DR out-proj of one 128-row st chunk."""
        st = b * 16 + qc * 4 + qt
        h8, l8 = c8_map[(b, qc)]
        ost = outp.tile([128, DM], BF16, tag="ost", bufs=3, name=f"ost_{st}")
        for chk in range(4):
            po = psum.tile([128, 512], F32, tag="pa", bufs=2,
                           name=f"po_{st}_{chk}")
            terms = ((h8, woh_sb), (h8, wol_sb), (l8, woh_sb))
            for n, (a8, w8) in enumerate(terms):
                nc.tensor.matmul(
                    po[:], a8[:, :, qt * 128:(qt + 1) * 128],
                    w8[:, :, chk * 512:(chk + 1) * 512],
                    start=(n == 0), stop=(n == 2), perf_mode=DR)
            nc.vector.tensor_scalar_mul(ost[:, chk * 512:(chk + 1) * 512],
                                        po[:], 1.0 / 4096.0)
            if chk % 2 == 1:
                yield 680
        nc.sync.dma_start(out[st * 128:(st + 1) * 128, :], ost[:])
        if (b, qc) in c8_map and qt == 3:
            del c8_map[(b, qc)]
        yield 30

    def c_st_bf16(b, qc, qt):
        """bf16 out-proj for the tail (last qc): the scores psum banks are
        free after the final exp, so borrow the sc tag for 4 banks of
        runway and do wide 1024-col copies split across DVE and ACT."""
        st = b * 16 + qc * 4 + qt
        cx = cxs_map[(b, qc)]
        ost = outp.tile([128, DM], BF16, tag="ost", bufs=3, name=f"ost_{st}")
        for half in range(2):
            po = psum.tile([128, 2, 512], F32, tag="sc", bufs=2,
                           name=f"pol_{st}_{half}")
            for sub in range(2):
                chk = half * 2 + sub
                for i in range(2):
                    nc.tensor.matmul(
                        po[:, sub, :], cx[:, i, qt * 128:(qt + 1) * 128],
                        wo_sb[:, i, chk * 512:(chk + 1) * 512],
                        start=(i == 0), stop=(i == 1))
            sl = ost[:, half * 1024:(half + 1) * 1024]
            if half == 1:
                nc.scalar.mul(sl, po[:], 1.0 / 64.0)
            else:
                nc.vector.tensor_scalar_mul(sl, po[:], 1.0 / 64.0)
            nc.sync.dma_start(
                out[st * 128:(st + 1) * 128, half * 1024:(half + 1) * 1024],
                sl)

    def split_c8(b, qc):
        """Pool-engine hi/lo fp8 split of this (b,qc)'s bf16 ctxT slice."""
        cx = cxs_map[(b, qc)]
        h8 = c8p.tile([128, 2, 512], FP8, tag="c8h", bufs=3,
                      name=f"c8h_{b}_{qc}")
        l8 = c8p.tile([128, 2, 512], FP8, tag="c8l", bufs=3,
                      name=f"c8l_{b}_{qc}")
        up = c8u.tile([128, 2, 512], BF16, tag="c8u", bufs=1,
                      name=f"c8u_{b}_{qc}")
        nc.gpsimd.tensor_copy(h8[:], cx[:])
        nc.gpsimd.tensor_copy(up[:], h8[:])
        nc.gpsimd.tensor_tensor(l8[:], cx[:], up[:], SUB)
        c8_map[(b, qc)] = (h8, l8)
        del cxs_map[(b, qc)]

    def ctx_gen(b, qc, h, ex):
        """ctx [q, 64+1] with exp tile stationary; normalization (with the
        x64 from the 1/64 ones col) fused into the psum->sbuf mul."""
        i, j = h // 2, h % 2
        if j == 0:
            cn_map[(b, qc, i)] = [
                smal.tile([128, 2, DH], BF16, tag="cn", bufs=8,
                          name=f"cn_{b}_{qc}_{i}_{qt}") for qt in range(4)]
        cn = cn_map[(b, qc, i)]
        pcx = psum.tile([128, 4, DH + 1], F32, tag="pc", bufs=2,
                        name=f"pcx_{b}_{qc}_{h}")
        for qt2 in range(2):
            for qt in (2 * qt2, 2 * qt2 + 1):
                for kt in range(DT):
                    nc.tensor.matmul(
                        pcx[:, qt, :], ex[:, kt, qt * 128:(qt + 1) * 128],
                        vsb[:, b * 16 + kt, :],
                        start=(kt == 0), stop=(kt == DT - 1))
            yield 900
        rr = smal.tile([128, 4], F32, tag="rr", bufs=3, name=f"rr_{b}_{qc}_{h}")
        nc.vector.reciprocal(rr[:], pcx[:, :, DH])
        for qt in range(4):
            nc.vector.tensor_scalar_mul(
                cn[qt][:, j, :], pcx[:, qt, 0:DH], rr[:, qt:qt + 1])
        yield 80
        if j == 1:
            if (b, qc) not in cxs_map:
                cxs_map[(b, qc)] = ctxs.tile([128, 2, 512], BF16, tag="cxs",
                                             bufs=2, name=f"cxs_{b}_{qc}")
            cx = cxs_map[(b, qc)]
            for qt in range(4):
                ctp = psum.tile([128, 128], BF16, tag="pc", bufs=2,
                                name=f"ctp_{b}_{qc}_{i}_{qt}")
                nc.tensor.transpose(ctp[:], cn[qt][:], ident[:])
                nc.vector.tensor_copy(cx[:, i, qt * 128:(qt + 1) * 128],
                                      ctp[:])
                yield 140
            del cn_map[(b, qc, i)]
            if i == 1 and not (b == 1 and qc == 3):
                split_c8(b, qc)
                for qt in range(4):
                    work.append(c_st_gen(b, qc, qt))

    # j == 1 of ctx(b, qc, *, i=0) must also create cxs before transposes:
    # handled inside ctx_gen (cxs created lazily at first j==1).

    def scores_slot(b, qc, h, actions, late=False):
        """8 scores/exp steps; after step s run actions[s] (list) then pump.
        late=True budget-pumps to pre-drain the deque before the last slot."""
        m, j = h // 2, h % 2
        bo = b * S
        qoff = bo + qc * 512
        ex = expp.tile([128, DT, 512], BF16, tag="exp", bufs=4,
                       name=f"ex_{b}_{qc}_{h}")
        for kt2 in range(DT // 2):
            pss = psum.tile([128, 2, 512], F32, tag="sc", bufs=2,
                            name=f"pss_{b}_{qc}_{h}_{kt2}")
            for t in range(2):
                koff = bo + (2 * kt2 + t) * 128
                nc.tensor.matmul(
                    pss[:, t, :], ktd[j * DH:(j + 1) * DH, koff:koff + 128],
                    qtp[j * DH:(j + 1) * DH, m, qoff:qoff + 512])
            nc.scalar.activation(
                ex[:, 2 * kt2:2 * kt2 + 2, :], pss[:], AF.Exp, scale=SCALE)
            if kt2 < len(actions):
                for act in actions[kt2]:
                    act()
            if late:
                pump_budget(820)
            else:
                pump_one()
        pump_one()
        pump_one()
        return ex

    def last_slot(b, qc):
        """Final slot (1,3,3): chase qt0/qt2 ctx in two psum banks behind the
        exp stream; qt1/qt3 + 4 bf16 out-proj tiles after the last exp."""
        m, j, i = 1, 1, 1
        bo = b * S
        qoff = bo + qc * 512
        ex = expp.tile([128, DT, 512], BF16, tag="exp", bufs=4,
                       name=f"ex_{b}_{qc}_3f")
        pc0 = psum.tile([128, DH + 1], F32, tag="pa", bufs=2, name="pcl_q0")
        pc2 = psum.tile([128, DH + 1], F32, tag="pc", bufs=2, name="pcl_q2")
        pcq = {0: pc0, 2: pc2}

        def chase(qt, kt, p):
            nc.tensor.matmul(
                p[:], ex[:, kt, qt * 128:(qt + 1) * 128],
                vsb[:, b * 16 + kt, :],
                start=(kt == 0), stop=(kt == DT - 1))

        for kt2 in range(DT // 2):
            pss = psum.tile([128, 2, 512], F32, tag="sc", bufs=2,
                            name=f"pss_{b}_{qc}_3_{kt2}")
            for t in range(2):
                koff = bo + (2 * kt2 + t) * 128
                nc.tensor.matmul(
                    pss[:, t, :], ktd[j * DH:(j + 1) * DH, koff:koff + 128],
                    qtp[j * DH:(j + 1) * DH, m, qoff:qoff + 512])
            nc.scalar.activation(
                ex[:, 2 * kt2:2 * kt2 + 2, :], pss[:], AF.Exp, scale=SCALE)
            if kt2 >= 1:
                for qt in (0, 2):
                    chase(qt, 2 * kt2 - 2, pcq[qt])
                    chase(qt, 2 * kt2 - 1, pcq[qt])
            pump_one()
        while pump_one():
            pass
        for qt in (0, 2):
            chase(qt, DT - 2, pcq[qt])
            chase(qt, DT - 1, pcq[qt])
        pc1 = psum.tile([128, DH + 1], F32, tag="pa", bufs=2, name="pcl_q1")
        pc3 = psum.tile([128, DH + 1], F32, tag="pc", bufs=2, name="pcl_q3")
        pcq[1], pcq[3] = pc1, pc3
        for kt in range(DT):
            chase(1, kt, pc1)
            chase(3, kt, pc3)
        cn = cn_map[(b, qc, i)]
        cx = cxs_map[(b, qc)]
        for qt in range(4):
            rr = smal.tile([128, 1], F32, tag="rrl", bufs=4, name=f"rrl_{qt}")
            nc.vector.reciprocal(rr[:], pcq[qt][:, DH:DH + 1])
            nc.vector.tensor_scalar_mul(cn[qt][:, 1, :], pcq[qt][:, 0:DH],
                                        rr[:])
        for qt in range(4):
            ctp = psum.tile([128, 128], BF16, tag="pc", bufs=2,
                            name=f"ctpl_{qt}")
            nc.tensor.transpose(ctp[:], cn[qt][:], ident[:])
            nc.vector.tensor_copy(cx[:, 1, qt * 128:(qt + 1) * 128], ctp[:])
            c_st_bf16(b, qc, qt)
        del cn_map[(b, qc, i)]

    # ---- Lead-in -----------------------------------------------------------
    # SP DMA order = priority: q00+K(b0) gate the first exps; V(b0)+q01
    # stream behind them (first ctx is gated to slot 4); Wo after.
    k_chs = {0: k_dma(0, 0)}
    q00 = q_dma(0, 0)
    k_chs[1] = k_dma(0, 1)
    k_chs[2] = k_dma(0, 2)
    k_chs[3] = k_dma(0, 3)
    nc.sync.dma_start(wvh_sb[:], wvh)
    nc.sync.dma_start(wvl_sb[:], wvl)
    v_chs = {0: v_dma(0, 0)}
    v_chs[1] = v_dma(0, 1)
    q01 = q_dma(0, 1)
    v_chs[2] = v_dma(0, 2)
    v_chs[3] = v_dma(0, 3)
    nc.sync.dma_start(woh_sb[:], woh)
    nc.sync.dma_start(wol_sb[:], wol)
    nc.sync.dma_start(wo_sb[:], wo)

    run_gen(k_proj(0, 0, k_chs[0]))
    run_gen(q_proj(0, 0, q00))

    kp_gens = {i: k_proj(0, i, k_chs[i]) for i in (1, 2, 3)}
    vp_gens = {i: v_proj(0, i, v_chs[i]) for i in range(4)}
    q_chunks = {(0, 1): q01}
    q_projs = {(0, 1): q_proj(0, 1, q01)}

    pend = deque()
    kv_sched = {5: ('k', 0), 6: ('v', 0), 7: ('k', 1), 8: ('v', 1),
                9: ('k', 2), 10: ('v', 2), 11: ('k', 3), 12: ('v', 3)}
    # carry-forward forced actions: slot -> list of (step, action)
    carry = {}

    def defer(slot, step, act):
        carry.setdefault(slot, []).append((step, act))

    def mk_actions(b, qc, h, slot):
        acts = [[] for _ in range(8)]
        for step, act in carry.pop(slot, ()):
            acts[step].append(act)
        if b == 0 and qc == 0:
            if h == 0:
                # chase the K-chunk DMAs: chunk c emitted before kt2 step 2c
                for s, gi in ((0, 1), (2, 2), (4, 3)):
                    g = kp_gens[gi]
                    acts[s] = [advance(g)] * 9
                    acts[s + 1].append(advance(g))
                    acts[s + 1].append(advance(g))
                    work.append(g)
            elif h == 2:
                g = vp_gens[0]
                for s in range(5):
                    acts[s].append(advance(g))
                g2 = vp_gens[1]
                for s in range(5, 8):
                    acts[s].append(advance(g2))
                work.append(g2)
            elif h == 3:
                # vp2/vp3 ride the work deque at slot 4 (V lands ~45us)
                work.append(vp_gens[2])
                work.append(vp_gens[3])
                g3 = q_projs[(0, 1)]
                for s in range(4, 8):
                    acts[s].append(advance(g3))
                work.append(g3)
            return acts
        # q prefetch: DMA at h0; proj pieces forced at h2/h3 of same qc
        if h == 0:
            nxt = (b, qc + 1) if qc < 3 else ((1, 0) if b == 0 else None)
            if nxt is not None and nxt not in q_chunks:
                q_chunks[nxt] = q_dma(*nxt)
                g = q_proj(*nxt, q_chunks[nxt])
                q_projs[nxt] = g
                defer(slot + 2, 1, advance(g))
                defer(slot + 2, 4, advance(g))
                defer(slot + 3, 1, advance(g))
                defer(slot + 3, 4, advance(g))
                work.append(g)
        # b1 K/V prefetch: DMA per kv_sched slot, projection forced 2 later
        if b == 0 and slot in kv_sched:
            kind, qq = kv_sched[slot]
            if kind == 'k':
                k_chs[(1, qq)] = k_dma(1, qq)
                g = k_proj(1, qq, k_chs[(1, qq)])
                kp_gens[(1, qq)] = g
            else:
                v_chs[(1, qq)] = v_dma(1, qq)
                g = v_proj(1, qq, v_chs[(1, qq)])
                vp_gens[(1, qq)] = g
            for s in range(1, 8):
                defer(slot + 2, s, advance(g))
            work.append(g)
        return acts

    for b in range(B):
        for qc in range(4):
            for h in range(4):
                slot = b * 16 + qc * 4 + h
                if b == 1 and qc == 3 and h == 3:
                    while pend:
                        work.append(ctx_gen(*pend.popleft()))
                    last_slot(b, qc)
                    continue
                acts = mk_actions(b, qc, h, slot)
                # ctx scheduling: lag 2; gate (0,0,*) ctx to slot >= 4
                while pend and len(pend) >= 2 and slot >= 4:
                    work.append(ctx_gen(*pend.popleft()))
                if b == 1 and qc == 3 and h == 2:
                    # drain everything before the final slot
                    while pend:
                        work.append(ctx_gen(*pend.popleft()))
                ex = scores_slot(b, qc, h, acts, late=(slot >= 27))
                pend.append((b, qc, h, ex))
    while pump_one():
        pass


def _build():
    nc = bacc.Bacc("TRN2", target_bir_lowering=False, debug=False,
                   num_devices=NC)
    qTh = nc.dram_tensor("qTh", [B, DM, S], FP8, kind="ExternalInput")
    qTl = nc.dram_tensor("qTl", [B, DM, S], FP8, kind="ExternalInput")
    kTh = nc.dram_tensor("kTh", [B, DM, S], FP8, kind="ExternalInput")
    kTl = nc.dram_tensor("kTl", [B, DM, S], FP8, kind="ExternalInput")
    vTh = nc.dram_tensor("vTh", [B, DM, S], FP8, kind="ExternalInput")
    vTl = nc.dram_tensor("vTl", [B, DM, S], FP8, kind="ExternalInput")
    wqh = nc.dram_tensor("wqh", [128, DT, DQ], FP8, kind="ExternalInput")
    wql = nc.dram_tensor("wql", [128, DT, DQ], FP8, kind="ExternalInput")
    wkh = nc.dram_tensor("wkh", [128, DT, DH], FP8, kind="ExternalInput")
    wkl = nc.dram_tensor("wkl", [128, DT, DH], FP8, kind="ExternalInput")
    wvh = nc.dram_tensor("wvh", [128, DT, DH], FP8, kind="ExternalInput")
    wvl = nc.dram_tensor("wvl", [128, DT, DH], FP8, kind="ExternalInput")
    woh = nc.dram_tensor("woh", [128, 2, DM], FP8, kind="ExternalInput")
    wol = nc.dram_tensor("wol", [128, 2, DM], FP8, kind="ExternalInput")
    wo = nc.dram_tensor("wo", [128, 2, DM], BF16, kind="ExternalInput")
    out = nc.dram_tensor("out", [BS, DM], BF16, kind="ExternalOutput")
    with tile.TileContext(nc) as tc:
        with ExitStack() as ctx:
            _emit(ctx, tc, qTh.ap(), qTl.ap(), kTh.ap(), kTl.ap(), vTh.ap(),
                  vTl.ap(), wqh.ap(), wql.ap(), wkh.ap(), wkl.ap(), wvh.ap(),
                  wvl.ap(), woh.ap(), wol.ap(), wo.ap(), out.ap())
    nc.compile()
    return nc


def _make_runner(nc, n_cores=NC):
    """Build the sharded jit callable once; reuse across kernel() calls."""
    bass2jax.install_neuronx_cc_hook()
    partition_name = (nc.partition_id_tensor.name
                      if nc.partition_id_tensor else None)
    in_names, out_names, out_avals, zero_outs = [], [], [], []
    for alloc in nc.m.functions[0].allocations:
        if not isinstance(alloc, mybir.MemoryLocationSet):
            continue
        name = alloc.memorylocations[0].name
        if alloc.kind == "ExternalInput":
            if name != partition_name:
                in_names.append(name)
        elif alloc.kind == "ExternalOutput":
            out_names.append(name)
            shape = tuple(alloc.tensor_shape)
            dtype = mybir.dt.np(alloc.dtype)
            out_avals.append(jax.core.ShapedArray(shape, dtype))
            zero_outs.append(np.zeros(shape, dtype))
    n_params = len(in_names)
    n_outs = len(out_avals)
    in_names_all = in_names + out_names
    if partition_name is not None:
        in_names_all.append(partition_name)
    donate = tuple(range(n_params, n_params + n_outs))

    def _body(*args):
        operands = list(args)
        if partition_name is not None:
            operands.append(bass2jax.partition_id_tensor())
        outs = bass2jax._bass_exec_p.bind(
            *operands,
            out_avals=tuple(out_avals),
            in_names=tuple(in_names_all),
            out_names=tuple(out_names),
            lowering_input_output_aliases=(),
            sim_require_finite=True,
            sim_require_nnan=True,
            nc=nc,
        )
        return tuple(outs)

    devices = jax.devices()[:n_cores]
    mesh = Mesh(np.asarray(devices), ("core",))
    in_specs = (PartitionSpec("core"),) * (n_params + n_outs)
    out_specs = (PartitionSpec("core"),) * len(out_names)
    sharded = jax.jit(
        shard_map(_body, mesh=mesh, in_specs=in_specs, out_specs=out_specs,
                  check_rep=False),
        donate_argnums=donate, keep_unused=True)
    sh = NamedSharding(mesh, PartitionSpec("core"))
    return sharded, in_names, out_names, zero_outs, sh


def _run(in_maps):
    if "nc" not in _cache:
        _cache["nc"] = _build()
    if "runner" not in _cache:
        _cache["runner"] = _make_runner(_cache["nc"])
    sharded, in_names, out_names, zero_outs, sh = _cache["runner"]
    n = NC
    concat_in = [
        jax.device_put(
            np.concatenate([np.asarray(in_maps[c][nm]) for c in range(n)], 0),
            sh)
        for nm in in_names
    ]
    zeros = [
        jax.device_put(np.zeros((n * z.shape[0], *z.shape[1:]), z.dtype), sh)
        for z in zero_outs
    ]
    outs = sharded(*concat_in, *zeros)
    i = out_names.index("out")
    arr = np.asarray(outs[i])           # [NC*BS, DM]
    return arr.reshape(n, BS, DM)


def _sbuf_layout(w):
    """[DM, X] -> [128, DT, X] (partition-major, dt tiles in free dim)."""
    x = w.shape[1]
    return np.ascontiguousarray(
        w.reshape(DT, 128, x).transpose(1, 0, 2))


def kernel(q, k, v, Wq, Wk, Wv, Wo):
    q = np.asarray(q, dtype=np.float32)
    k = np.asarray(k, dtype=np.float32)
    v = np.asarray(v, dtype=np.float32)
    bf = ml_dtypes.bfloat16
    f8 = ml_dtypes.float8_e4m3
    qT32 = np.ascontiguousarray(q.transpose(0, 2, 1))
    qThi = qT32.astype(f8)
    qTlo = (qT32 - qThi.astype(np.float32)).astype(f8)
    kT32 = np.ascontiguousarray(k.transpose(0, 2, 1))
    kThi = kT32.astype(f8)
    kTlo = (kT32 - kThi.astype(np.float32)).astype(f8)
    vT32 = np.ascontiguousarray(v.transpose(0, 2, 1))
    vThi = vT32.astype(f8)
    vTlo = (vT32 - vThi.astype(np.float32)).astype(f8)
    Wq64 = np.asarray(Wq, dtype=np.float32) * 64.0
    Wqhi = Wq64.astype(f8)
    Wqlo = (Wq64 - Wqhi.astype(np.float32)).astype(f8)
    Wk64 = np.asarray(Wk, dtype=np.float32) * 64.0
    Wkhi = Wk64.astype(f8)
    Wklo = (Wk64 - Wkhi.astype(np.float32)).astype(f8)
    Wv64 = np.asarray(Wv, dtype=np.float32) * 64.0
    Wvhi = Wv64.astype(f8)
    Wvlo = (Wv64 - Wvhi.astype(np.float32)).astype(f8)
    Wo32 = np.asarray(Wo, dtype=np.float32)
    Wo64 = Wo32 * 64.0

    def wo_layout(w):
        # [DQ, DM] -> [128, 2, DM]: dq = i*128 + p
        return np.ascontiguousarray(
            w.reshape(2, 128, DM).transpose(1, 0, 2))

    in_maps = []
    for c in range(NC):
        woc64 = Wo64[c * DQ:(c + 1) * DQ, :]
        wochi = woc64.astype(f8)
        woclo = (woc64 - wochi.astype(np.float32)).astype(f8)
        in_maps.append({
            "qTh": qThi, "qTl": qTlo, "kTh": kThi, "kTl": kTlo,
            "vTh": vThi, "vTl": vTlo,
            "wqh": _sbuf_layout(Wqhi[:, c * DQ:(c + 1) * DQ]),
            "wql": _sbuf_layout(Wqlo[:, c * DQ:(c + 1) * DQ]),
            "wkh": _sbuf_layout(Wkhi[:, c * DH:(c + 1) * DH]),
            "wkl": _sbuf_layout(Wklo[:, c * DH:(c + 1) * DH]),
            "wvh": _sbuf_layout(Wvhi[:, c * DH:(c + 1) * DH]),
            "wvl": _sbuf_layout(Wvlo[:, c * DH:(c + 1) * DH]),
            "woh": wo_layout(wochi),
            "wol": wo_layout(woclo),
            "wo": wo_layout(Wo32[c * DQ:(c + 1) * DQ, :].astype(bf)),
        })
    partials = _run(in_maps)
    out = partials.astype(np.float32, copy=False).sum(axis=0)
    return out.reshape(B, S, DM)


# revision 41
# speedup vs baseline: 1.0007x; 1.0007x over previous
"""GQA multi-head attention (B=2, S=2048, D=2048, 32 q-heads / 8 kv-heads)
on 8 Trainium2 NeuronCores.

Sharding: tensor-parallel over kv-head groups. Core c owns kv head c and its
4 query heads: Wq column-shard [2048, 256], Wk/Wv column-shard [2048, 64],
Wo row-shard [256, 2048]. Each core computes a full-shape partial output
(its heads' contribution through Wo); the host sums the 8 partials.

Numerics: dense projections (q/k/v and the out-projection) run as fp8e4
DoubleRow matmuls with hi+lo error compensation: x ~ hi + lo in fp8,
out = xh*Wh + xh*Wl + xl*Wh (0.75x the bf16 PE cost; dropped lo*lo term is
~2^-8 relative). Host pre-scales weights x64 so fp8 lo parts don't denormal;
ctx is scaled x64 on-device (the vsb ones column is 1/64 so the softmax
reciprocal supplies the 64); 1/4096 is folded into the out-proj psum copies.
The ctx hi/lo split for the fp8 out-proj runs on the idle Pool engine
(bit-exact vs host split; verified on the NEFF path). The last (b=1,qc=3)
out-proj stays bf16 so the end tail has no Pool dependency.

Schedule (the kernel is ACT-exp bound at ~266us; PE busy ~275us):
- Lead-in is DMA-ordered: wk, wq, q(0,0), K(0,0..3) first; scores slot 0
  chases the K-chunk DMAs at kt granularity. V(b0) + q(0,1) + Wo stream
  behind them; V-projections are pumped as filler in slots 1-4 (first ctx
  is gated to slot 4 when V(b0) has landed).
- Per-slot round-robin of small PE pieces (0.3-1.3us) between the 8
  scores/exp steps keeps the ACT queue fed; ctx lags scores by 2 slots
  (exp quadruple-buffered to absorb the V-gated start), out-proj pieces
  run eagerly as soon as each (b,qc)'s ctx is split to fp8.
- Tail: the last slot chases ctx for qt0/qt2 in two psum banks at kt
  granularity behind the exp stream; qt1/qt3 + the 4 bf16 out-proj tiles
  are all that remains after the final exp.

DMA XBAR transpose races on the compiled NEFF path - PE transposes only.
"""
from collections import deque
from contextlib import ExitStack

import numpy as np
import ml_dtypes

import jax

try:
    jax.config.update("jax_compilation_cache_dir", "/tmp/jax_bass_cache")
    jax.config.update("jax_persistent_cache_min_compile_time_secs", 1.0)
except Exception:
    pass

from jax.sharding import Mesh, PartitionSpec, NamedSharding
from jax.experimental.shard_map import shard_map

import concourse.bass as bass
import concourse.mybir as mybir
import concourse.tile as tile
from concourse import bacc, bass2jax
from concourse.masks import make_identity

BF16 = mybir.dt.bfloat16
FP8 = mybir.dt.float8e4
F32 = mybir.dt.float32
AF = mybir.ActivationFunctionType
DR = mybir.MatmulPerfMode.DoubleRow
SUB = mybir.AluOpType.subtract

B, S, DM = 2, 2048, 2048
HKV, G, DH = 8, 4, 64
DQ = G * DH            # 256: per-core q-projection width
NC = 8
DT = DM // 128         # 16 contraction tiles
BS = B * S             # 4096
SCALE = 1.0 / 8.0      # 1/sqrt(64)

_cache = {}


def _emit(ctx, tc, qTh, qTl, kTh, kTl, vTh, vTl, wqh, wql, wkh, wkl, wvh,
          wvl, woh, wol, wo, out):
    nc = tc.nc

    pp = ctx.enter_context(tc.tile_pool(name="persist", bufs=1))
    wqh_sb = pp.tile([128, DT, DQ], FP8, tag="wqh")
    wql_sb = pp.tile([128, DT, DQ], FP8, tag="wql")
    wkh_sb = pp.tile([128, DT, DH], FP8, tag="wkh")
    wkl_sb = pp.tile([128, DT, DH], FP8, tag="wkl")
    wvh_sb = pp.tile([128, DT, DH], FP8, tag="wvh")
    wvl_sb = pp.tile([128, DT, DH], FP8, tag="wvl")
    woh_sb = pp.tile([128, 2, DM], FP8, tag="woh")
    wol_sb = pp.tile([128, 2, DM], FP8, tag="wol")
    wo_sb = pp.tile([128, 2, DM], BF16, tag="wo")
    qtp = pp.tile([128, 2, BS], BF16, tag="qtp")    # QT pairs [p, m, b*S+s]
    ktd = pp.tile([128, BS], BF16, tag="ktd")       # KT duplicated both halves
    vsb = pp.tile([128, BS // 128, DH + 1], BF16, tag="vsb")  # V + 1/64 col
    ident = pp.tile([128, 128], BF16, tag="ident")
    make_identity(nc, ident[:])
    nc.gpsimd.memset(vsb[:, :, DH], 1.0 / 64.0)

    kst = ctx.enter_context(tc.tile_pool(name="kst", bufs=4))
    vst = ctx.enter_context(tc.tile_pool(name="vst", bufs=4))
    qst = ctx.enter_context(tc.tile_pool(name="qst", bufs=3))
    expp = ctx.enter_context(tc.tile_pool(name="expp", bufs=4))
    ctxs = ctx.enter_context(tc.tile_pool(name="ctxs", bufs=2))
    c8p = ctx.enter_context(tc.tile_pool(name="c8p", bufs=3))
    c8u = ctx.enter_context(tc.tile_pool(name="c8u", bufs=1))
    smal = ctx.enter_context(tc.tile_pool(name="small", bufs=2))
    outp = ctx.enter_context(tc.tile_pool(name="outp", bufs=3))
    psum = ctx.enter_context(tc.tile_pool(name="psum", bufs=1, space="PSUM"))

    # ---- weight DMAs (SP queue; order = priority) --------------------------
    nc.sync.dma_start(wkh_sb[:], wkh)
    nc.sync.dma_start(wkl_sb[:], wkl)
    nc.sync.dma_start(wqh_sb[:], wqh)
    nc.sync.dma_start(wql_sb[:], wql)

    # ---- work-piece pump ---------------------------------------------------
    # Generators yield their nominal PE cost (ns); the pump meters emission
    # so the exp stream is never starved and no backlog dumps into the tail.
    work = deque()

    def pump_one():
        while work:
            try:
                next(work[0])
                return True
            except StopIteration:
                work.popleft()
        return False

    def pump_budget(budget):
        spent = 0
        while work and spent < budget:
            try:
                c = next(work[0])
                spent += c if c else 400
            except StopIteration:
                work.popleft()
        return spent

    def run_gen(g):
        for _ in g:
            pass

    def advance(g):
        return lambda: next(g, None)

    # ---- DMA emitters ------------------------------------------------------
    # K/V stage in half-chunks (dt 0-7 / 8-15) so the a-half frees mid-gen
    # and the next chunk's DMA streams behind the projection.
    def k_dma(b, qc):
        so = qc * 512
        chs = []
        for half in range(2):
            for nm, src in (("kh", kTh), ("kl", kTl)):
                t = kst.tile([128, DT // 2, 512], FP8, tag="kst",
                             name=f"{nm}{half}_{b}_{qc}")
                nc.sync.dma_start(
                    t[:],
                    src[b].rearrange("(dt p) s -> p dt s", p=128)
                    [:, 8 * half:8 * half + 8, so:so + 512])
                chs.append(t)
        return chs

    def v_dma(b, qc):
        so = qc * 512
        chs = []
        for half in range(2):
            for nm, src in (("vh", vTh), ("vl", vTl)):
                t = vst.tile([128, DT // 2, 512], FP8, tag="vst",
                             name=f"{nm}{half}_{b}_{qc}")
                nc.sync.dma_start(
                    t[:],
                    src[b].rearrange("(dt p) s -> p dt s", p=128)
                    [:, 8 * half:8 * half + 8, so:so + 512])
                chs.append(t)
        return chs

    def q_dma(b, qc):
        so = qc * 512
        chs = []
        for nm, src in (("qh", qTh), ("ql", qTl)):
            t = qst.tile([128, DT, 512], FP8, tag="qst", name=f"{nm}_{b}_{qc}")
            nc.sync.dma_start(
                t[:],
                src[b].rearrange("(dt p) s -> p dt s", p=128)[:, :, so:so + 512])
            chs.append(t)
        return chs

    # ---- projection generators --------------------------------------------
    def _kv_matmuls(dst, chs, wh_sb, wl_sb):
        """ki-major 3-term DR accumulation over both stage halves: one psum
        accumulation group open at a time."""
        h_a, l_a, h_b, l_b = chs
        for ki in range(4):
            n = 0
            for half, (hh, ll) in enumerate(((h_a, l_a), (h_b, l_b))):
                for chv, w_sb in ((hh, wh_sb), (hh, wl_sb), (ll, wh_sb)):
                    for t in range(4):
                        wt = 8 * half + 2 * t
                        nc.tensor.matmul(
                            dst[:, ki, :],
                            chv[:, 2 * t:2 * t + 2, ki * 128:(ki + 1) * 128],
                            w_sb[:, wt:wt + 2, :],
                            start=(n == 0), stop=(n == 23),
                            perf_mode=DR)
                        n += 1
            yield 340

    def k_proj(b, qc, chs):
        """K chunk -> ktd (transposed, duplicated to both halves)."""
        bo, so = b * S, qc * 512
        kp = psum.tile([128, 4, DH], F32, tag="pa", bufs=2, name=f"kp_{b}_{qc}")
        yield from _kv_matmuls(kp, chs, wkh_sb, wkl_sb)
        ktmp = smal.tile([128, 4, DH], BF16, tag="ktmp", bufs=2,
                         name=f"ktmp_{b}_{qc}")
        nc.vector.tensor_scalar_mul(ktmp[:], kp[:], 1.0 / 64.0)
        yield 60
        for ki in range(4):
            koff = bo + so + ki * 128
            ktp = psum.tile([128, 128], BF16, tag="pc", bufs=2,
                            name=f"ktp_{b}_{qc}_{ki}")
            for half in range(2):
                nc.tensor.transpose(
                    ktp[DH * half:DH * half + DH, :], ktmp[:, ki, :],
                    ident[:], tile_position=(0, DH * half))
            nc.vector.tensor_copy(ktd[:, koff:koff + 128], ktp[:])
            if ki % 2 == 1:
                yield 280

    def v_proj(b, qc, chs):
        """V chunk -> vsb rows (keys-major, 1/64 ones col preset)."""
        vp = psum.tile([128, 4, DH], F32, tag="pa", bufs=2, name=f"vp_{b}_{qc}")
        yield from _kv_matmuls(vp, chs, wvh_sb, wvl_sb)
        nc.vector.tensor_scalar_mul(
            vsb[:, b * 16 + qc * 4:b * 16 + qc * 4 + 4, 0:DH], vp[:],
            1.0 / 64.0)
        yield 60

    def q_proj(b, qc, chs):
        qh_ch, ql_ch = chs
        bo, so = b * S, qc * 512
        terms = [(wqh_sb, qh_ch), (wqh_sb, ql_ch), (wql_sb, qh_ch)]
        for m in range(2):
            pq = psum.tile([128, 512], F32, tag="pa", bufs=2,
                           name=f"pq_{b}_{qc}_{m}")
            n = 0
            for w_sb, qch in terms:
                for t in range(DT // 2):
                    nc.tensor.matmul(
                        pq[:], w_sb[:, 2 * t:2 * t + 2, m * 128:(m + 1) * 128],
                        qch[:, 2 * t:2 * t + 2, :],
                        start=(n == 0), stop=(n == 3 * DT // 2 - 1),
                        perf_mode=DR)
                    n += 1
                    if n == 12:
                        yield 640
            nc.vector.tensor_scalar_mul(
                qtp[:, m, bo + so:bo + so + 512], pq[:], 1.0 / 64.0)
            yield 660

    # ---- ctx / out-proj ----------------------------------------------------
    cn_map = {}
    c8_map = {}     # (b, qc) -> (hi fp8, lo fp8) [128, 2, 512]
    cxs_map = {}    # (b, qc) -> bf16 ctxT staging [128, 2, 512]

    def c_st_gen(b, qc, qt):
        """fp8 3-term DR out-proj of one 128-row st chunk."""
        st = b * 16 + qc * 4 + qt
        h8, l8 = c8_map[(b, qc)]
        ost = outp.tile([128, DM], BF16, tag="ost", bufs=3, name=f"ost_{st}")
        for chk in range(4):
            po = psum.tile([128, 512], F32, tag="pa", bufs=2,
                           name=f"po_{st}_{chk}")
            terms = ((h8, woh_sb), (h8, wol_sb), (l8, woh_sb))
            for n, (a8, w8) in enumerate(terms):
                nc.tensor.matmul(
                    po[:], a8[:, :, qt * 128:(qt + 1) * 128],
                    w8[:, :, chk * 512:(chk + 1) * 512],
                    start=(n == 0), stop=(n == 2), perf_mode=DR)
            nc.vector.tensor_scalar_mul(ost[:, chk * 512:(chk + 1) * 512],
                                        po[:], 1.0 / 4096.0)
            if chk % 2 == 1:
                yield 680
        nc.sync.dma_start(out[st * 128:(st + 1) * 128, :], ost[:])
        if (b, qc) in c8_map and qt == 3:
            del c8_map[(b, qc)]
        yield 30

    def c_st_bf16(b, qc, qt):
        """bf16 out-proj for the tail (last qc): the scores psum banks are
        free after the final exp, so borrow the sc tag for 4 banks of
        runway and do wide 1024-col copies split across DVE and ACT."""
        st = b * 16 + qc * 4 + qt
        cx = cxs_map[(b, qc)]
        ost = outp.tile([128, DM], BF16, tag="ost", bufs=3, name=f"ost_{st}")
        for half in range(2):
            po = psum.tile([128, 2, 512], F32, tag="sc", bufs=2,
                           name=f"pol_{st}_{half}")
            for sub in range(2):
                chk = half * 2 + sub
                for i in range(2):
                    nc.tensor.matmul(
                        po[:, sub, :], cx[:, i, qt * 128:(qt + 1) * 128],
                        wo_sb[:, i, chk * 512:(chk + 1) * 512],
                        start=(i == 0), stop=(i == 1))
            sl = ost[:, half * 1024:(half + 1) * 1024]
            if half == 1:
                nc.scalar.mul(sl, po[:], 1.0 / 64.0)
            else:
                nc.vector.tensor_scalar_mul(sl, po[:], 1.0 / 64.0)
            nc.sync.dma_start(
                out[st * 128:(st + 1) * 128, half * 1024:(half + 1) * 1024],
                sl)

    def split_c8(b, qc):
        """Pool-engine hi/lo fp8 split of this (b,qc)'s bf16 ctxT slice."""
        cx = cxs_map[(b, qc)]
        h8 = c8p.tile([128, 2, 512], FP8, tag="c8h", bufs=3,
                      name=f"c8h_{b}_{qc}")
        l8 = c8p.tile([128, 2, 512], FP8, tag="c8l", bufs=3,
                      name=f"c8l_{b}_{qc}")
        up = c8u.tile([128, 2, 512], BF16, tag="c8u", bufs=1,
                      name=f"c8u_{b}_{qc}")
        nc.gpsimd.tensor_copy(h8[:], cx[:])
        nc.gpsimd.tensor_copy(up[:], h8[:])
        nc.gpsimd.tensor_tensor(l8[:], cx[:], up[:], SUB)
        c8_map[(b, qc)] = (h8, l8)
        del cxs_map[(b, qc)]

    def ctx_gen(b, qc, h, ex):
        """ctx [q, 64+1] with exp tile stationary; normalization (with the
        x64 from the 1/64 ones col) fused into the psum->sbuf mul."""
        i, j = h // 2, h % 2
        if j == 0:
            cn_map[(b, qc, i)] = [
                smal.tile([128, 2, DH], BF16, tag="cn", bufs=8,
                          name=f"cn_{b}_{qc}_{i}_{qt}") for qt in range(4)]
        cn = cn_map[(b, qc, i)]
        pcx = psum.tile([128, 4, DH + 1], F32, tag="pc", bufs=2,
                        name=f"pcx_{b}_{qc}_{h}")
        for qt2 in range(2):
            for qt in (2 * qt2, 2 * qt2 + 1):
                for kt in range(DT):
                    nc.tensor.matmul(
                        pcx[:, qt, :], ex[:, kt, qt * 128:(qt + 1) * 128],
                        vsb[:, b * 16 + kt, :],
                        start=(kt == 0), stop=(kt == DT - 1))
            yield 900
        rr = smal.tile([128, 4], F32, tag="rr", bufs=3, name=f"rr_{b}_{qc}_{h}")
        nc.vector.reciprocal(rr[:], pcx[:, :, DH])
        for qt in range(4):
            nc.vector.tensor_scalar_mul(
                cn[qt][:, j, :], pcx[:, qt, 0:DH], rr[:, qt:qt + 1])
        yield 80
        if j == 1:
            if (b, qc) not in cxs_map:
                cxs_map[(b, qc)] = ctxs.tile([128, 2, 512], BF16, tag="cxs",
                                             bufs=2, name=f"cxs_{b}_{qc}")
            cx = cxs_map[(b, qc)]
            for qt in range(4):
                ctp = psum.tile([128, 128], BF16, tag="pc", bufs=2,
                                name=f"ctp_{b}_{qc}_{i}_{qt}")
                nc.tensor.transpose(ctp[:], cn[qt][:], ident[:])
                nc.vector.tensor_copy(cx[:, i, qt * 128:(qt + 1) * 128],
                                      ctp[:])
                yield 140
            del cn_map[(b, qc, i)]
            if i == 1 and not (b == 1 and qc == 3):
                split_c8(b, qc)
                for qt in range(4):
                    work.append(c_st_gen(b, qc, qt))

    # j == 1 of ctx(b, qc, *, i=0) must also create cxs before transposes:
    # handled inside ctx_gen (cxs created lazily at first j==1).

    def scores_slot(b, qc, h, actions, late=False):
        """8 scores/exp steps; after step s run actions[s] (list) then pump.
        late=True budget-pumps to pre-drain the deque before the last slot."""
        m, j = h // 2, h % 2
        bo = b * S
        qoff = bo + qc * 512
        ex = expp.tile([128, DT, 512], BF16, tag="exp", bufs=4,
                       name=f"ex_{b}_{qc}_{h}")
        for kt2 in range(DT // 2):
            pss = psum.tile([128, 2, 512], F32, tag="sc", bufs=2,
                            name=f"pss_{b}_{qc}_{h}_{kt2}")
            for t in range(2):
                koff = bo + (2 * kt2 + t) * 128
                nc.tensor.matmul(
                    pss[:, t, :], ktd[j * DH:(j + 1) * DH, koff:koff + 128],
                    qtp[j * DH:(j + 1) * DH, m, qoff:qoff + 512])
            nc.scalar.activation(
                ex[:, 2 * kt2:2 * kt2 + 2, :], pss[:], AF.Exp, scale=SCALE)
            if kt2 < len(actions):
                for act in actions[kt2]:
                    act()
            if late:
                pump_budget(620)
            else:
                pump_one()
        pump_one()
        pump_one()
        return ex

    def last_slot(b, qc):
        """Final slot (1,3,3): chase qt0/qt2 ctx in two psum banks behind the
        exp stream; qt1/qt3 + 4 bf16 out-proj tiles after the last exp."""
        m, j, i = 1, 1, 1
        bo = b * S
        qoff = bo + qc * 512
        ex = expp.tile([128, DT, 512], BF16, tag="exp", bufs=4,
                       name=f"ex_{b}_{qc}_3f")
        pc0 = psum.tile([128, DH + 1], F32, tag="pa", bufs=2, name="pcl_q0")
        pc2 = psum.tile([128, DH + 1], F32, tag="pc", bufs=2, name="pcl_q2")
        pcq = {0: pc0, 2: pc2}

        def chase(qt, kt, p):
            nc.tensor.matmul(
                p[:], ex[:, kt, qt * 128:(qt + 1) * 128],
                vsb[:, b * 16 + kt, :],
                start=(kt == 0), stop=(kt == DT - 1))

        for kt2 in range(DT // 2):
            pss = psum.tile([128, 2, 512], F32, tag="sc", bufs=2,
                            name=f"pss_{b}_{qc}_3_{kt2}")
            for t in range(2):
                koff = bo + (2 * kt2 + t) * 128
                nc.tensor.matmul(
                    pss[:, t, :], ktd[j * DH:(j + 1) * DH, koff:koff + 128],
                    qtp[j * DH:(j + 1) * DH, m, qoff:qoff + 512])
            nc.scalar.activation(
                ex[:, 2 * kt2:2 * kt2 + 2, :], pss[:], AF.Exp, scale=SCALE)
            if kt2 >= 1:
                for qt in (0, 2):
                    chase(qt, 2 * kt2 - 2, pcq[qt])
                    chase(qt, 2 * kt2 - 1, pcq[qt])
            pump_one()
        while pump_one():
            pass
        for qt in (0, 2):
            chase(qt, DT - 2, pcq[qt])
            chase(qt, DT - 1, pcq[qt])
        pc1 = psum.tile([128, DH + 1], F32, tag="pa", bufs=2, name="pcl_q1")
        pc3 = psum.tile([128, DH + 1], F32, tag="pc", bufs=2, name="pcl_q3")
        pcq[1], pcq[3] = pc1, pc3
        for kt in range(DT):
            chase(1, kt, pc1)
            chase(3, kt, pc3)
        cn = cn_map[(b, qc, i)]
        cx = cxs_map[(b, qc)]
        for qt in range(4):
            rr = smal.tile([128, 1], F32, tag="rrl", bufs=4, name=f"rrl_{qt}")
            nc.vector.reciprocal(rr[:], pcq[qt][:, DH:DH + 1])
            nc.vector.tensor_scalar_mul(cn[qt][:, 1, :], pcq[qt][:, 0:DH],
                                        rr[:])
        for qt in range(4):
            ctp = psum.tile([128, 128], BF16, tag="pc", bufs=2,
                            name=f"ctpl_{qt}")
            nc.tensor.transpose(ctp[:], cn[qt][:], ident[:])
            nc.vector.tensor_copy(cx[:, 1, qt * 128:(qt + 1) * 128], ctp[:])
            c_st_bf16(b, qc, qt)
        del cn_map[(b, qc, i)]

    # ---- Lead-in -----------------------------------------------------------
    # SP DMA order = priority: q00+K(b0) gate the first exps; V(b0)+q01
    # stream behind them (first ctx is gated to slot 4); Wo after.
    k_chs = {0: k_dma(0, 0)}
    q00 = q_dma(0, 0)
    k_chs[1] = k_dma(0, 1)
    k_chs[2] = k_dma(0, 2)
    k_chs[3] = k_dma(0, 3)
    nc.sync.dma_start(wvh_sb[:], wvh)
    nc.sync.dma_start(wvl_sb[:], wvl)
    v_chs = {0: v_dma(0, 0)}
    v_chs[1] = v_dma(0, 1)
    q01 = q_dma(0, 1)
    v_chs[2] = v_dma(0, 2)
    v_chs[3] = v_dma(0, 3)
    nc.sync.dma_start(woh_sb[:], woh)
    nc.sync.dma_start(wol_sb[:], wol)
    nc.sync.dma_start(wo_sb[:], wo)

    run_gen(k_proj(0, 0, k_chs[0]))
    run_gen(q_proj(0, 0, q00))

    kp_gens = {i: k_proj(0, i, k_chs[i]) for i in (1, 2, 3)}
    vp_gens = {i: v_proj(0, i, v_chs[i]) for i in range(4)}
    q_chunks = {(0, 1): q01}
    q_projs = {(0, 1): q_proj(0, 1, q01)}

    pend = deque()
    kv_sched = {5: ('k', 0), 6: ('v', 0), 7: ('k', 1), 8: ('v', 1),
                9: ('k', 2), 10: ('v', 2), 11: ('k', 3), 12: ('v', 3)}
    # carry-forward forced actions: slot -> list of (step, action)
    carry = {}

    def defer(slot, step, act):
        carry.setdefault(slot, []).append((step, act))

    def mk_actions(b, qc, h, slot):
        acts = [[] for _ in range(8)]
        for step, act in carry.pop(slot, ()):
            acts[step].append(act)
        if b == 0 and qc == 0:
            if h == 0:
                # chase the K-chunk DMAs: chunk c emitted before kt2 step 2c
                for s, gi in ((0, 1), (2, 2), (4, 3)):
                    g = kp_gens[gi]
                    acts[s] = [advance(g)] * 9
                    acts[s + 1].append(advance(g))
                    acts[s + 1].append(advance(g))
                    work.append(g)
            elif h == 2:
                g = vp_gens[0]
                for s in range(5):
                    acts[s].append(advance(g))
                g2 = vp_gens[1]
                for s in range(5, 8):
                    acts[s].append(advance(g2))
                work.append(g2)
            elif h == 3:
                # vp2/vp3 ride the work deque at slot 4 (V lands ~45us)
                work.append(vp_gens[2])
                work.append(vp_gens[3])
                g3 = q_projs[(0, 1)]
                for s in range(4, 8):
                    acts[s].append(advance(g3))
                work.append(g3)
            return acts
        # q prefetch: DMA at h0; proj pieces forced at h2/h3 of same qc
        if h == 0:
            nxt = (b, qc + 1) if qc < 3 else ((1, 0) if b == 0 else None)
            if nxt is not None and nxt not in q_chunks:
                q_chunks[nxt] = q_dma(*nxt)
                g = q_proj(*nxt, q_chunks[nxt])
                q_projs[nxt] = g
                defer(slot + 2, 1, advance(g))
                defer(slot + 2, 4, advance(g))
                defer(slot + 3, 1, advance(g))
                defer(slot + 3, 4, advance(g))
                work.append(g)
        # b1 K/V prefetch: DMA per kv_sched slot, projection forced 2 later
        if b == 0 and slot in kv_sched:
            kind, qq = kv_sched[slot]
            if kind == 'k':
                k_chs[(1, qq)] = k_dma(1, qq)
                g = k_proj(1, qq, k_chs[(1, qq)])
                kp_gens[(1, qq)] = g
            else:
                v_chs[(1, qq)] = v_dma(1, qq)
                g = v_proj(1, qq, v_chs[(1, qq)])
                vp_gens[(1, qq)] = g
            for s in range(1, 8):
                defer(slot + 2, s, advance(g))
            work.append(g)
        return acts

    for b in range(B):
        for qc in range(4):
            for h in range(4):
                slot = b * 16 + qc * 4 + h
                if b == 1 and qc == 3 and h == 3:
                    while pend:
                        work.append(ctx_gen(*pend.popleft()))
                    last_slot(b, qc)
                    continue
                acts = mk_actions(b, qc, h, slot)
                # ctx scheduling: lag 2; gate (0,0,*) ctx to slot >= 4
                while pend and len(pend) >= 2 and slot >= 4:
                    work.append(ctx_gen(*pend.popleft()))
                if b == 1 and qc == 3 and h == 2:
                    # drain everything before the final slot
                    while pend:
                        work.append(ctx_gen(*pend.popleft()))
                ex = scores_slot(b, qc, h, acts, late=(slot >= 27))
                pend.append((b, qc, h, ex))
    while pump_one():
        pass


def _build():
    nc = bacc.Bacc("TRN2", target_bir_lowering=False, debug=False,
                   num_devices=NC)
    qTh = nc.dram_tensor("qTh", [B, DM, S], FP8, kind="ExternalInput")
    qTl = nc.dram_tensor("qTl", [B, DM, S], FP8, kind="ExternalInput")
    kTh = nc.dram_tensor("kTh", [B, DM, S], FP8, kind="ExternalInput")
    kTl = nc.dram_tensor("kTl", [B, DM, S], FP8, kind="ExternalInput")
    vTh = nc.dram_tensor("vTh", [B, DM, S], FP8, kind="ExternalInput")
    vTl = nc.dram_tensor("vTl", [B, DM, S], FP8, kind="ExternalInput")
    wqh = nc.dram_tensor("wqh", [128, DT, DQ], FP8, kind="ExternalInput")
    wql = nc.dram_tensor("wql", [128, DT, DQ], FP8, kind="ExternalInput")
    wkh = nc.dram_tensor("wkh", [128, DT, DH], FP8, kind="ExternalInput")
    wkl = nc.dram_tensor("wkl", [128, DT, DH], FP8, kind="ExternalInput")
    wvh = nc.dram_tensor("wvh", [128, DT, DH], FP8, kind="ExternalInput")
    wvl = nc.dram_tensor("wvl", [128, DT, DH], FP8, kind="ExternalInput")
    woh = nc.dram_tensor("woh", [128, 2, DM], FP8, kind="ExternalInput")
    wol = nc.dram_tensor("wol", [128, 2, DM], FP8, kind="ExternalInput")
    wo = nc.dram_tensor("wo", [128, 2, DM], BF16, kind="ExternalInput")
    out = nc.dram_tensor("out", [BS, DM], BF16, kind="ExternalOutput")
    with tile.TileContext(nc) as tc:
        with ExitStack() as ctx:
            _emit(ctx, tc, qTh.ap(), qTl.ap(), kTh.ap(), kTl.ap(), vTh.ap(),
                  vTl.ap(), wqh.ap(), wql.ap(), wkh.ap(), wkl.ap(), wvh.ap(),
                  wvl.ap(), woh.ap(), wol.ap(), wo.ap(), out.ap())
    nc.compile()
    return nc


def _make_runner(nc, n_cores=NC):
    """Build the sharded jit callable once; reuse across kernel() calls."""
    bass2jax.install_neuronx_cc_hook()
    partition_name = (nc.partition_id_tensor.name
                      if nc.partition_id_tensor else None)
    in_names, out_names, out_avals, zero_outs = [], [], [], []
    for alloc in nc.m.functions[0].allocations:
        if not isinstance(alloc, mybir.MemoryLocationSet):
            continue
        name = alloc.memorylocations[0].name
        if alloc.kind == "ExternalInput":
            if name != partition_name:
                in_names.append(name)
        elif alloc.kind == "ExternalOutput":
            out_names.append(name)
            shape = tuple(alloc.tensor_shape)
            dtype = mybir.dt.np(alloc.dtype)
            out_avals.append(jax.core.ShapedArray(shape, dtype))
            zero_outs.append(np.zeros(shape, dtype))
    n_params = len(in_names)
    n_outs = len(out_avals)
    in_names_all = in_names + out_names
    if partition_name is not None:
        in_names_all.append(partition_name)
    donate = tuple(range(n_params, n_params + n_outs))

    def _body(*args):
        operands = list(args)
        if partition_name is not None:
            operands.append(bass2jax.partition_id_tensor())
        outs = bass2jax._bass_exec_p.bind(
            *operands,
            out_avals=tuple(out_avals),
            in_names=tuple(in_names_all),
            out_names=tuple(out_names),
            lowering_input_output_aliases=(),
            sim_require_finite=True,
            sim_require_nnan=True,
            nc=nc,
        )
        return tuple(outs)

    devices = jax.devices()[:n_cores]
    mesh = Mesh(np.asarray(devices), ("core",))
    in_specs = (PartitionSpec("core"),) * (n_params + n_outs)
    out_specs = (PartitionSpec("core"),) * len(out_names)
    sharded = jax.jit(
        shard_map(_body, mesh=mesh, in_specs=in_specs, out_specs=out_specs,
                  check_rep=False),
        donate_argnums=donate, keep_unused=True)
    sh = NamedSharding(mesh, PartitionSpec("core"))
    return sharded, in_names, out_names, zero_outs, sh


def _run(in_maps):
    if "nc" not in _cache:
        _cache["nc"] = _build()
    if "runner" not in _cache:
        _cache["runner"] = _make_runner(_cache["nc"])
    sharded, in_names, out_names, zero_outs, sh = _cache["runner"]
    n = NC
    concat_in = [
        jax.device_put(
            np.concatenate([np.asarray(in_maps[c][nm]) for c in range(n)], 0),
            sh)
        for nm in in_names
    ]
    zeros = [
        jax.device_put(np.zeros((n * z.shape[0], *z.shape[1:]), z.dtype), sh)
        for z in zero_outs
    ]
    outs = sharded(*concat_in, *zeros)
    i = out_names.index("out")
    arr = np.asarray(outs[i])           # [NC*BS, DM]
    return arr.reshape(n, BS, DM)


def _sbuf_layout(w):
    """[DM, X] -> [128, DT, X] (partition-major, dt tiles in free dim)."""
    x = w.shape[1]
    return np.ascontiguousarray(
        w.reshape(DT, 128, x).transpose(1, 0, 2))


def kernel(q, k, v, Wq, Wk, Wv, Wo):
    q = np.asarray(q, dtype=np.float32)
    k = np.asarray(k, dtype=np.float32)
    v = np.asarray(v, dtype=np.float32)
    bf = ml_dtypes.bfloat16
    f8 = ml_dtypes.float8_e4m3
    qT32 = np.ascontiguousarray(q.transpose(0, 2, 1))
    qThi = qT32.astype(f8)
    qTlo = (qT32 - qThi.astype(np.float32)).astype(f8)
    kT32 = np.ascontiguousarray(k.transpose(0, 2, 1))
    kThi = kT32.astype(f8)
    kTlo = (kT32 - kThi.astype(np.float32)).astype(f8)
    vT32 = np.ascontiguousarray(v.transpose(0, 2, 1))
    vThi = vT32.astype(f8)
    vTlo = (vT32 - vThi.astype(np.float32)).astype(f8)
    Wq64 = np.asarray(Wq, dtype=np.float32) * 64.0
    Wqhi = Wq64.astype(f8)
    Wqlo = (Wq64 - Wqhi.astype(np.float32)).astype(f8)
    Wk64 = np.asarray(Wk, dtype=np.float32) * 64.0
    Wkhi = Wk64.astype(f8)
    Wklo = (Wk64 - Wkhi.astype(np.float32)).astype(f8)
    Wv64 = np.asarray(Wv, dtype=np.float32) * 64.0
    Wvhi = Wv64.astype(f8)
    Wvlo = (Wv64 - Wvhi.astype(np.float32)).astype(f8)
    Wo32 = np.asarray(Wo, dtype=np.float32)
    Wo64 = Wo32 * 64.0

    def wo_layout(w):
        # [DQ, DM] -> [128, 2, DM]: dq = i*128 + p
        return np.ascontiguousarray(
            w.reshape(2, 128, DM).transpose(1, 0, 2))

    in_maps = []
    for c in range(NC):
        woc64 = Wo64[c * DQ:(c + 1) * DQ, :]
        wochi = woc64.astype(f8)
        woclo = (woc64 - wochi.astype(np.float32)).astype(f8)
        in_maps.append({
            "qTh": qThi, "qTl": qTlo, "kTh": kThi, "kTl": kTlo,
            "vTh": vThi, "vTl": vTlo,
            "wqh": _sbuf_layout(Wqhi[:, c * DQ:(c + 1) * DQ]),
            "wql": _sbuf_layout(Wqlo[:, c * DQ:(c + 1) * DQ]),
            "wkh": _sbuf_layout(Wkhi[:, c * DH:(c + 1) * DH]),
            "wkl": _sbuf_layout(Wklo[:, c * DH:(c + 1) * DH]),
            "wvh": _sbuf_layout(Wvhi[:, c * DH:(c + 1) * DH]),
            "wvl": _sbuf_layout(Wvlo[:, c * DH:(c + 1) * DH]),
            "woh": wo_layout(wochi),
            "wol": wo_layout(woclo),
            "wo": wo_layout(Wo32[c * DQ:(c + 1) * DQ, :].astype(bf)),
        })
    partials = _run(in_maps)
    out = partials.astype(np.float32, copy=False).sum(axis=0)
    return out.reshape(B, S, DM)
